# revision 1
# baseline (speedup 1.0000x reference)
"""DiceLoss kernel: PE computes BOTH intersect and most sum-of-squares via
the diagonal trick.

Per class, DVE builds the mask (tensor_scalar 4x); PE accumulates
mask^T @ x chunk blocks into a per-stat 512B PSUM slot whose diagonal is
the intersect partial. For pe-square classes, PE also accumulates
x^T @ x blocks (diag = sum-of-squares partial). A 258ns DVE STT against an
identity tile extracts each diagonal (lagged so DVE never waits on fresh
PE output). PSUM slots are assigned round-robin with a bank-striding
pattern; each stat's first chunk zeroes its own slot via start=True, so
slot reuse needs no memset. Remaining squares run fused on ACT.
"""
import numpy as np
import ml_dtypes
import concourse.bacc as bacc
import concourse.mybir as mybir
import concourse.tile as tile
from concourse.bass_utils import run_bass_kernel_spmd

N_CORES = 8
B, C, X, Y, Z = 2, 33, 96, 96, 96
XS = X // N_CORES
VOX = XS * Y * Z
P = 128
F = VOX // P
FB = B * F                   # 1728
SMOOTH = 1e-5
CH = [(j * 128, 128) for j in range(13)] + [(13 * 128, 64)]
NP_ = (C + 1) // 2           # 17 class-pairs (class 33 is zero padding)

_cached = {}


def _build(pe_sq=None, lag=2):
    if pe_sq is None:
        pe_sq = {1, 4, 7, 10, 13, 16, 19, 22, 25, 28, 31}
    nc = bacc.Bacc("TRN2", target_bir_lowering=False, debug=False,
                   num_devices=N_CORES)
    dt = mybir.dt.bfloat16
    f32 = mybir.dt.float32
    x_in = nc.dram_tensor("x", [NP_, P, 2 * FB], dt, kind="ExternalInput")
    lab_in = nc.dram_tensor("lab", [P, FB], dt, kind="ExternalInput")
    stats = nc.dram_tensor("stats", [2, P, C], f32, kind="ExternalOutput")
    pairs = [(2 * i, 2) for i in range(C // 2)] + [(C - 1, 1)]
    with tile.TileContext(nc) as tc:
        with (
            tc.tile_pool(name="xp", bufs=4) as xp,
            tc.tile_pool(name="labp", bufs=1) as labp,
            tc.tile_pool(name="maskp", bufs=6) as maskp,
            tc.tile_pool(name="scrd", bufs=6) as scrdp,
            tc.tile_pool(name="scr2", bufs=3) as scr2p,
            tc.tile_pool(name="stat", bufs=1) as statp,
            tc.tile_pool(name="psum", bufs=1, space="PSUM") as psp,
        ):
            lab_t = labp.tile([P, FB], dt)
            nc.sync.dma_start(lab_t[:], lab_in[:, :])
            iota_t = statp.tile([P, P], mybir.dt.int32, tag="iota")
            nc.gpsimd.iota(iota_t[:], pattern=[[1, P]], base=0,
                           channel_multiplier=-1)
            ident = statp.tile([P, P], f32, tag="ident")
            nc.vector.tensor_scalar(ident[:], iota_t[:], 0, None,
                                    mybir.AluOpType.is_equal)
            int_pp = statp.tile([P, C], f32, tag="int")
            sq_pp = statp.tile([P, C], f32, tag="sq")
            int_ps = psp.tile([P, 4096], f32)
            stat_ctr = [0]
            pending = []               # (slot, dest_tile, col)

            def emit_matmuls(lhs_ap, rhs_ap_of, dest, col):
                k = stat_ctr[0] % 32
                slot = (k % 8) * 4 + k // 8
                stat_ctr[0] += 1
                off = slot * 128
                for j, (o, m) in enumerate(CH):
                    nc.tensor.matmul(
                        int_ps[0:m, off:off + m],
                        lhs_ap[:, o:o + m],
                        rhs_ap_of[:, o:o + m],
                        start=(j == 0), stop=False, skip_group_check=True)
                pending.append((off, dest, col))
                if len(pending) > lag:
                    emit_diag(*pending.pop(0))

            def emit_diag(off, dest, col):
                scrd = scrdp.tile([P, P], f32)
                nc.vector.scalar_tensor_tensor(
                    out=scrd[:], in0=int_ps[0:P, off:off + P],
                    scalar=0.0, in1=ident[:],
                    op0=mybir.AluOpType.bypass, op1=mybir.AluOpType.mult,
                    accum_out=dest[:, col:col + 1])

            for c0, n in pairs:
                pp = c0 // 2
                if n == 1:
                    # last class: skip the zero padding half
                    xt = xp.tile([P, FB], dt, tag="xt_last")
                    nc.sync.dma_start(xt[:], x_in[pp, :, 0:FB])
                elif pp == 0:
                    # first pair: quartered load so compute starts early
                    xt = xp.tile([P, 2 * FB], dt)
                    qw = FB // 2
                    for qi in range(4):
                        nc.sync.dma_start(
                            xt[:, qi * qw:(qi + 1) * qw],
                            x_in[pp, :, qi * qw:(qi + 1) * qw])
                else:
                    xt = xp.tile([P, 2 * FB], dt)
                    nc.sync.dma_start(xt[:], x_in[pp, :, :])
                for qq in range(n):
                    c = c0 + qq
                    xs = xt[:, qq * FB:(qq + 1) * FB]
                    mask = maskp.tile([P, FB], dt)
                    nc.vector.tensor_scalar(mask[:], lab_t[:], float(c), None,
                                            mybir.AluOpType.is_equal)
                    emit_matmuls(mask, xs, int_pp, c)
                    if c in pe_sq:
                        emit_matmuls(xs, xs, sq_pp, c)
                    else:
                        scr2 = scr2p.tile([P, FB], dt)
                        nc.scalar.activation(
                            out=scr2[:], in_=xs,
                            func=mybir.ActivationFunctionType.Square,
                            accum_out=sq_pp[:, c:c + 1])
            for args in pending:
                emit_diag(*args)
            nc.sync.dma_start(stats[0, :, :], int_pp[:])
            nc.sync.dma_start(stats[1, :, :], sq_pp[:])
    nc.compile()
    return nc


def _get_nc():
    if "nc" not in _cached:
        _cached["nc"] = _build()
    return _cached["nc"]


def kernel(outputs, label):
    nc = _get_nc()
    outputs = np.asarray(outputs)
    lab_np = np.asarray(label)
    bf16 = ml_dtypes.bfloat16
    in_maps = []
    for k in range(N_CORES):
        xs = outputs[:, :, k * XS:(k + 1) * XS].reshape(B, C, P, F)
        xs = np.ascontiguousarray(xs.transpose(1, 2, 0, 3)).reshape(C, P, FB)
        xpad = np.zeros((2 * NP_, P, FB), xs.dtype)
        xpad[:C] = xs
        xs = xpad.reshape(NP_, 2, P, FB).transpose(0, 2, 1, 3).reshape(
            NP_, P, 2 * FB)
        ls = lab_np[:, k * XS:(k + 1) * XS].reshape(B, P, F)
        ls = np.ascontiguousarray(ls.transpose(1, 0, 2)).reshape(P, FB)
        in_maps.append({"x": xs.astype(bf16), "lab": ls.astype(bf16)})
    res = run_bass_kernel_spmd(nc, in_maps, core_ids=list(range(N_CORES)))
    intersect = np.zeros(C, np.float64)
    sumsq = np.zeros(C, np.float64)
    for r in res.results:
        st = r["stats"].astype(np.float64)
        intersect += st[0].sum(axis=0)
        sumsq += st[1].sum(axis=0)
    labels_sum = np.bincount(
        lab_np.reshape(-1).astype(np.int64), minlength=C).astype(np.float64)
    dice = (2.0 * intersect + SMOOTH) / (sumsq + labels_sum + SMOOTH)
    return np.float32(np.mean(1.0 - dice))



# revision 6
# speedup vs baseline: 1.2045x; 1.2045x over previous
"""DiceLoss kernel: fp8 inputs + DoubleRow PE matmuls via the diagonal trick.

x is quantized to fp8e4m3 on the host (halves HBM traffic; DMA is the
roofline at ~21.6us/core). Per class a bf16 mask (lab==c)*2^-15 is built
in one tensor_scalar (4x DVE mode for most classes, gpsimd for a few to
balance load); the bf16 tile's odd bytes, bitcast to fp8e4, are exactly
1.0/0.0 and feed the PE as the DoubleRow stationary at 0.5 cyc/row. Per
class the PE accumulates an intersect block (mask^T @ x) and a squares
block (x^T @ x) into adjacent 64-col PSUM regions at partition base 0
(the ISA rejects other dst partitions at 128-row contraction). Extraction
is pipelined across engines so DVE stays mask-bound: ACT copies the
128-col class slot PSUM->SBUF (gpsimd cannot read PSUM, and SBUF operands
give DVE its 2x mode), then two DVE ident-multiply STTs lagged three
classes behind write the diagonals' row-sums into stats[:, c]. The 32
128-col class slots tile PSUM exactly; class 32 reuses slot 0 long after
its extract. x loads are grouped 4 classes per DMA so HWDGE overhead
stays off the critical path. Host sums partials and applies the ratio.
"""
import numpy as np
import ml_dtypes
import concourse.bacc as bacc
import concourse.mybir as mybir
import concourse.tile as tile
from concourse.bass_utils import run_bass_kernel_spmd

N_CORES = 8
B, C, X, Y, Z = 2, 33, 96, 96, 96
XS = X // N_CORES
VOX = XS * Y * Z
P = 128
F = VOX // P
FB = B * F                   # 1728
KT = FB // 64                # 27 k-tiles of 64 cols
SMOOTH = 1e-5
MASK_LO = float(2.0 ** -15)  # bf16 0x3800: odd byte 0x38 == fp8e4 1.0
POOL_MASKS = {5, 11, 17, 23, 29, 32}

_cached = {}


def _build():
    nc = bacc.Bacc("TRN2", target_bir_lowering=False, debug=False,
                   num_devices=N_CORES)
    bf16 = mybir.dt.bfloat16
    fp8 = mybir.dt.float8e4
    f32 = mybir.dt.float32
    DR = mybir.MatmulPerfMode.DoubleRow
    x_in = nc.dram_tensor("x", [C, P, FB], fp8, kind="ExternalInput")
    lab_in = nc.dram_tensor("lab", [P, KT, 64], bf16, kind="ExternalInput")
    sel_in = nc.dram_tensor("sel", [64, 64], f32, kind="ExternalInput")
    stats = nc.dram_tensor("stats", [P, C], f32, kind="ExternalOutput")
    with tile.TileContext(nc) as tc:
        with (
            tc.tile_pool(name="xp", bufs=1) as xp,
            tc.tile_pool(name="labp", bufs=1) as labp,
            tc.tile_pool(name="maskp", bufs=6) as maskp,
            tc.tile_pool(name="pmaskp", bufs=len(POOL_MASKS)) as pmaskp,
            tc.tile_pool(name="scr", bufs=4) as scrp,
            tc.tile_pool(name="stat", bufs=1) as statp,
            tc.tile_pool(name="psum", bufs=1, space="PSUM") as psp,
        ):
            lab_t = labp.tile([P, KT, 64], bf16)
            nc.sync.dma_start(lab_t[:], lab_in[:, :, :])
            sel_t = statp.tile([64, 64], f32, tag="sel")
            nc.sync.dma_start(sel_t[:], sel_in[:, :])
            stat_t = statp.tile([P, C], f32, tag="stat")
            xt = xp.tile([P, C, KT, 64], fp8)
            groups = [(0, 1)] + [(1 + 4 * g, 4) for g in range(8)]
            for c0, n in groups:
                nc.sync.dma_start(xt[:, c0:c0 + n, :, :],
                                  x_in[c0:c0 + n, :, :])
            ps = psp.tile([P, 4096], f32)
            pending = []

            def emit_extract(c, scr):
                nc.vector.scalar_tensor_tensor(
                    out=scr[:, 0:64], in0=scr[:, 0:64], scalar=0.0,
                    in1=sel_t[:], op0=mybir.AluOpType.bypass,
                    op1=mybir.AluOpType.mult,
                    accum_out=stat_t[0:64, c:c + 1])
                nc.vector.scalar_tensor_tensor(
                    out=scr[:, 64:128], in0=scr[:, 64:128], scalar=0.0,
                    in1=sel_t[:], op0=mybir.AluOpType.bypass,
                    op1=mybir.AluOpType.mult,
                    accum_out=stat_t[64:128, c:c + 1])

            for c in range(C):
                if c in POOL_MASKS:
                    mt = pmaskp.tile([P, KT, 64], bf16)
                    eng = nc.gpsimd
                else:
                    mt = maskp.tile([P, KT, 64], bf16)
                    eng = nc.vector
                eng.tensor_scalar(mt[:], lab_t[:], float(c), MASK_LO,
                                  mybir.AluOpType.is_equal,
                                  mybir.AluOpType.mult)
                m8 = mt[:].bitcast(fp8)          # [P, KT, 128]; odd lanes
                xc = xt[:, c]                    # [P, KT, 64]
                o = (c % 32) * 128
                for j in range(13):
                    k = 2 * j
                    nc.tensor.matmul(
                        ps[0:64, o:o + 64], m8[:, k:k + 2, 1::2],
                        xc[:, k:k + 2, :], start=(j == 0), stop=False,
                        perf_mode=DR, skip_group_check=True)
                nc.tensor.matmul(
                    ps[0:64, o:o + 64], m8[:, 26:27, 1::2], xc[:, 26, :],
                    start=False, stop=True, skip_group_check=True)
                for j in range(13):
                    k = 2 * j
                    nc.tensor.matmul(
                        ps[0:64, o + 64:o + 128], xc[:, k:k + 2, :],
                        xc[:, k:k + 2, :], start=(j == 0), stop=False,
                        perf_mode=DR, skip_group_check=True)
                nc.tensor.matmul(
                    ps[0:64, o + 64:o + 128], xc[:, 26, :], xc[:, 26, :],
                    start=False, stop=True, skip_group_check=True)
                scr = scrp.tile([64, 128], f32)
                nc.scalar.activation(
                    out=scr[:], in_=ps[0:64, o:o + 128],
                    func=mybir.ActivationFunctionType.Copy)
                pending.append((c, scr))
                if len(pending) > 3:
                    emit_extract(*pending.pop(0))
            for args in pending:
                emit_extract(*args)
            nc.sync.dma_start(stats[:, :], stat_t[:])
    nc.compile()
    return nc


def _get_nc():
    if "nc" not in _cached:
        _cached["nc"] = _build()
    return _cached["nc"]


def kernel(outputs, label):
    nc = _get_nc()
    outputs = np.asarray(outputs)
    lab_np = np.asarray(label)
    fp8 = ml_dtypes.float8_e4m3
    bf16 = ml_dtypes.bfloat16
    sel = np.eye(64, dtype=np.float32)
    in_maps = []
    for k in range(N_CORES):
        xs = outputs[:, :, k * XS:(k + 1) * XS].reshape(B, C, P, F)
        xs = np.ascontiguousarray(xs.transpose(1, 2, 0, 3)).reshape(C, P, FB)
        ls = lab_np[:, k * XS:(k + 1) * XS].reshape(B, P, F)
        ls = np.ascontiguousarray(ls.transpose(1, 0, 2)).reshape(P, KT, 64)
        in_maps.append({"x": xs.astype(fp8), "lab": ls.astype(bf16),
                        "sel": sel})
    res = run_bass_kernel_spmd(nc, in_maps, core_ids=list(range(N_CORES)))
    intersect = np.zeros(C, np.float64)
    sumsq = np.zeros(C, np.float64)
    for r in res.results:
        st = r["stats"].astype(np.float64)
        intersect += st[:64].sum(axis=0)
        sumsq += st[64:].sum(axis=0)
    labels_sum = np.bincount(
        lab_np.reshape(-1).astype(np.int64), minlength=C).astype(np.float64)
    dice = (2.0 * intersect + SMOOTH) / (sumsq + labels_sum + SMOOTH)
    return np.float32(np.mean(1.0 - dice))
